# revision 60
# baseline (speedup 1.0000x reference)
"""Single-head causal attention (B=4, T=2048, C=1024) on 8 trn2 NeuronCores.

Associativity-folded fp8 (e4m3) DoubleRow rewrite. Exploits two host-side
weight foldings so the C x C projections act on the SHARDED query side
(1024 rows/core) instead of the DUPLICATED kv side (2048 rows/core):

  softmax row-shift invariance  ->  bk drops out entirely
  S = (x_q M + u) x^T           with M = s Wq^T Wk,  u = s Wk^T bq
  y = (A x) G^T / den + beff    with G = Wp Wv,      beff = b_proj + Wp bv

K, V, Q, O never materialize. Per-core tensor work becomes
  Qt-proj (49k) + scores (58k) + rowsum (5k) + AX (61k) + z-proj (49k)
= 223k cost-model cycles (~93us) vs the baseline's 420k (~175us).

All matmul groups stay 3-term hi/lo fp8 DoubleRow (a*b ~= ah*bh + ah*bl +
al*bh) — the cheapest >=7-bit-effective scheme under the cost model.

Sharding: identical to baseline: 8 shards = (batch b) x (query interleave h),
query rows as interleaved 256-row blocks; S^T formulation (scores [kv, query])
so softmax denominators come from ones-matmuls and AX needs no transposes.

Scales (power-of-2):
  x*2^2 hi/lo;  M*2^12 hi/lo;  Qt stored *2^7 (u*2^7 bias);  S psum = S*2^9
  A = exp(S + EXPB) hi + boosted residual Al*2^5 (paired with xt5 = xth*2^-5
  and a 2^-5 ones tile);  AX psum = AX*2^2, stored AX*2^-1 hi + (res*2^4)
  paired with G5 = G*2^3;  G*2^7 hi/lo;  z psum = z*2^6;
  y = psum * (2^2/den) * 2^-8 + beff.

AX accumulation is psum-direct (no SBUF staging): phase B2 computes q-cols
[0,512) (kv half 0 is the only contributor there — causal interleave), phase
D2 computes q-cols [512,1024) with full contraction over both kv halves.
"""

import sys

sys.path.insert(0, "/opt/trn_rl_repo")

import numpy as np
import ml_dtypes

import concourse.bass as bass
import concourse.tile as tile
from concourse import mybir
from concourse.vector_clock import ScopedClock

FP = mybir.dt.float32
BF = mybir.dt.bfloat16
F8 = mybir.dt.float8e4
E4 = ml_dtypes.float8_e4m3
AF = mybir.ActivationFunctionType

P = 128
C = 1024  # embed dim
H = 1024  # query rows per core
NT = C // P  # 8 channel tiles
NP = 4  # channel pair-tiles
NEG = -1.0e12
EXPB = -3.11  # exp bias: A = exp(score + EXPB); max score 7.42 -> A_max ~74

_MAX_WAITS = 1


class _TC(tile.TileContext):
    """TileContext whose tail drain puts its global-clock waits on a nop
    (walrus rejects multi-wait Drain); excess waits are split by
    _split_waits() afterwards."""

    def _drain_and_barrier(self, tick_clock, wait_clock):
        nop_inst = self.nc.sync.nop(nofuse=True, hint="pre_drain_waits")
        wait_clock.add_sem_waits(
            nop_inst.ins, ScopedClock({None: tick_clock.global_clock})
        )
        self.nc.sync.drain()
        self.nc.all_engine_barrier()
        assert self.sems is not None
        popped = self.nc._tile_sem_poison_stack.pop()
        assert popped is self._sem_poison
        self.nc.clear_and_free_semaphores(list(self.sems.allocated().values()))


def _split_waits(nc, max_waits=_MAX_WAITS):
    """Move excess sync waits onto injected nops placed immediately before
    the instruction on the same engine (walrus rejects >1 wait/instruction)."""
    import copy

    template = nc.sync.nop(nofuse=True, hint="waitsplit_template").ins
    counter = [0]

    def make_nop(engine, waits):
        nop = copy.deepcopy(template)
        counter[0] += 1
        nop.name = f"I-wsplit-{counter[0]}"
        nop.engine = engine
        nop.sync_info = mybir.SyncInfo(on_wait=list(waits), on_update=[])
        return nop

    f = nc.m.functions[0]
    for bb in f.blocks:
        insts = bb.instructions
        if not any(
            i.sync_info and i.sync_info.on_wait and len(i.sync_info.on_wait) > max_waits
            for i in insts
        ):
            continue
        newlist = []
        for inst in insts:
            si = inst.sync_info
            if si and si.on_wait and len(si.on_wait) > max_waits:
                if inst.name == template.name:
                    newlist.append(inst)
                    continue
                waits = list(si.on_wait)
                del si.on_wait[max_waits:]
                rest = waits[max_waits:]
                while rest:
                    newlist.append(make_nop(inst.engine, rest[:max_waits]))
                    rest = rest[max_waits:]
            newlist.append(inst)
        bb.instructions[:] = newlist


def _chunks(lo, hi, step=512):
    out = []
    while lo < hi:
        w = min(step, hi - lo)
        out.append((lo, lo + w))
        lo += w
    return out


# Interleaved-256 balanced causal structure (see baseline docstring).
SLO = [0, 0, 0, 1, 2, 2, 2, 3]  # per kv 128-tile: first valid 128-col block
MASKNAMES = [
    [(0, "m1d")],
    [(0, "m1f"), (1, "m1d")],
    [(0, "m2d"), (1, "m1f")],
    [(1, "m2d")],
    [(2, "m1d")],
    [(2, "m1f"), (3, "m1d")],
    [(2, "m2d"), (3, "m1f")],
    [(3, "m2d")],
]
PLO = [0, 0, 2, 2]  # per kv-pair js: min(SLO[2js], SLO[2js+1])
DR = mybir.MatmulPerfMode.DoubleRow


def _build_nc():
    nc = bass.Bass("TRN2", target_bir_lowering=False, debug=False)

    def din(name, shape, dt=F8):
        return nc.dram_tensor(name, shape, dt, kind="ExternalInput").ap()

    # x packs, c-layout (contraction = channel): [128, NP*2*1024] j-major
    xqh, xql = din("xqh", [P, NP, 2, H]), din("xql", [P, NP * 2 * H])
    xoh, xol = din("xoh", [P, NP * 2 * H]), din("xol", [P, NP * 2 * H])
    xxh, xxl = din("xxh", [P, NP * 2 * H]), din("xxl", [P, NP * 2 * H])
    # x packs, kv-layout (contraction = kv token): for AX lhsT
    xtoh, xtol = din("xtoh", [P, NP * 2 * C]), din("xtol", [P, NP * 2 * C])
    xtxh, xtxl = din("xtxh", [P, NP * 2 * C]), din("xtxl", [P, NP * 2 * C])
    xto5, xtx5 = din("xto5", [P, NP * 2 * C]), din("xtx5", [P, NP * 2 * C])
    # folded weights
    mh, ml = din("mh", [P, NP, 2, C]), din("ml", [P, NP * 2 * C])
    gh, gl = din("gh", [P, NP * 2 * C]), din("gl", [P, NP * 2 * C])
    g5 = din("g5", [P, NP * 2 * C])
    ub = din("ub", [P, NT], FP)
    beff = din("beff", [P, NT], FP)
    ones_h_in = din("ones_h", [P, 2 * P])
    ones_l_in = din("ones_l", [P, 2 * P])
    m1d_in = din("m1d_in", [P, P], FP)
    m1f_in = din("m1f_in", [P, P], FP)
    m2d_in = din("m2d_in", [P, P], FP)
    # output in (o2-tile, chunk)-major layout; host reassembles
    yT = nc.dram_tensor("yT", [NT * 2 * P, 512], BF, kind="ExternalOutput").ap()

    with _TC(nc) as tc:
        with (
            tc.tile_pool(name="misc", bufs=1) as misc,
            tc.tile_pool(name="big", bufs=1) as big,
            tc.tile_pool(name="stg", bufs=5) as stg,
            tc.tile_pool(name="ostg", bufs=4) as ostg,
            tc.tile_pool(name="evac", bufs=4) as evac,
            tc.tile_pool(name="psum", bufs=8, space="PSUM") as pp,
        ):
            ones_h = misc.tile([P, 2, P], F8, tag="ones_h")
            ones_l = misc.tile([P, 2, P], F8, tag="ones_l")
            m1d = misc.tile([P, P], FP, tag="m1d")
            m1f = misc.tile([P, P], FP, tag="m1f")
            m2d = misc.tile([P, P], FP, tag="m2d")
            ub_sb = misc.tile([P, NT], FP, tag="ub")
            beff_sb = misc.tile([P, NT], FP, tag="beff")
            ebias = misc.tile([P, 1], FP, tag="ebias")
            zbias = misc.tile([P, 1], FP, tag="zbias")
            nc.gpsimd.memset(ebias[:], EXPB)
            nc.gpsimd.memset(zbias[:], 0.0)

            MT = {"m1d": m1d, "m1f": m1f, "m2d": m2d}

            # ---- persistent tensors ------------------------------------
            # Qt: [c-part, c-pair, q]  (rhs of scores)
            Qth = [big.tile([P, 2, H], F8, tag=f"Qth{j}", name=f"Qth{j}") for j in range(NP)]
            Qtl = [big.tile([P, 2, H], F8, tag=f"Qtl{j}", name=f"Qtl{j}") for j in range(NP)]
            # x c-layout (scores lhsT), 4 j-tiles per pack held as one tile
            txoh = big.tile([P, NP, 2, H], F8, tag="txoh", name="txoh")
            txol = big.tile([P, NP, 2, H], F8, tag="txol", name="txol")
            txxh = big.tile([P, NP, 2, H], F8, tag="txxh", name="txxh")
            txxl = big.tile([P, NP, 2, H], F8, tag="txxl", name="txxl")
            # x kv-layout (AX lhsT)
            txtoh = big.tile([P, NP, 2, C], F8, tag="txtoh", name="txtoh")
            txtol = big.tile([P, NP, 2, C], F8, tag="txtol", name="txtol")
            txtxh = big.tile([P, NP, 2, C], F8, tag="txtxh", name="txtxh")
            txtxl = big.tile([P, NP, 2, C], F8, tag="txtxl", name="txtxl")
            txto5 = big.tile([P, NP, 2, C], F8, tag="txto5", name="txto5")
            txtx5 = big.tile([P, NP, 2, C], F8, tag="txtx5", name="txtx5")
            # =============================================================
            # Phase A: Qt projection  (Qt = x_q M + u)
            # =============================================================
            with tc.tile_pool(name="aw", bufs=1) as aw:
                tmh = [aw.tile([P, 2, C], F8, tag=f"tmh{j}", name=f"tmh{j}") for j in range(NP)]
                tml = [aw.tile([P, 2, C], F8, tag=f"tml{j}", name=f"tml{j}") for j in range(NP)]
                txqh = [aw.tile([P, 2, H], F8, tag=f"txqh{j}", name=f"txqh{j}") for j in range(NP)]
                txql = [aw.tile([P, 2, H], F8, tag=f"txql{j}", name=f"txql{j}") for j in range(NP)]

                # DMA order matches matmul consumption: first operands as
                # slivers for an early start; tml rides each j-wave so every
                # arriving tile unlocks work in all 8 open psum groups
                # first two transfers on separate issue queues (SP + Act)
                # so their HWDGE setups overlap
                nc.sync.dma_start(tmh[0][:, :, 0:128], mh[:, 0, :, 0:128])
                nc.scalar.dma_start(txqh[0][:, :, 0:512], xqh[:, 0, :, 0:512])
                nc.sync.dma_start(tmh[0][:, :, 128:C], mh[:, 0, :, 128:C])
                nc.sync.dma_start(txqh[0][:, :, 512:H], xqh[:, 0, :, 512:H])
                nc.sync.dma_start(tml[0][:], ml[:, 0 : 2 * C])
                for j in range(1, NP):
                    nc.sync.dma_start(tmh[j][:], mh[:, j, :, :])
                    nc.sync.dma_start(txqh[j][:], xqh[:, j, :, :])
                    nc.sync.dma_start(tml[j][:], ml[:, j * 2 * C : (j + 1) * 2 * C])
                nc.sync.dma_start(ub_sb[:], ub[:])
                for j in range(NP):
                    nc.sync.dma_start(txql[j][:], xql[:, j * 2 * H : (j + 1) * 2 * H])
                # B1 operands trickle in behind the A-phase packs
                nc.sync.dma_start(ones_h[:], ones_h_in[:])
                nc.sync.dma_start(ones_l[:], ones_l_in[:])
                nc.sync.dma_start(m1d[:], m1d_in[:])
                nc.sync.dma_start(m1f[:], m1f_in[:])
                nc.sync.dma_start(m2d[:], m2d_in[:])
                nc.sync.dma_start(txoh[:], xoh[:])
                nc.sync.dma_start(txol[:], xol[:])

                sc = tc.nc.named_scope("A_qt"); sc.__enter__()
                for ci in range(2):
                    cs = ci * 512
                    for ot in range(NT):
                        colsl = slice(ot * P, (ot + 1) * P)
                        ps = pp.tile([P, 512], FP, tag="ps", name=f"psqt{ci}_{ot}")
                        # j-major over the M hi/lo terms to match the DMA
                        # wave order (tmh_j, txqh_j, tml_j), txql terms last
                        terms = [(tmh, txqh, j) for j in range(NP)]
                        terms = [t for j in range(NP)
                                 for t in ((tmh, txqh, j), (tml, txqh, j))]
                        terms += [(tmh, txql, j) for j in range(NP)]
                        for n, (wt, xt, j) in enumerate(terms):
                            nc.tensor.matmul(
                                ps[:],
                                lhsT=wt[j][:, :, colsl],
                                rhs=xt[j][:, :, cs : cs + 512],
                                start=(n == 0),
                                stop=(n == 11),
                                perf_mode=DR,
                            )
                        st = stg.tile([P, 512], FP, tag="st", name=f"stqt{ci}_{ot}")
                        qh_sl = Qth[ot // 2][:, ot % 2, cs : cs + 512]
                        nc.scalar.activation(
                            st[:], ps[:], AF.Identity,
                            bias=ub_sb[:, ot : ot + 1], scale=2.0**-7,
                        )
                        nc.gpsimd.tensor_copy(qh_sl, st[:])
                        nc.vector.tensor_sub(
                            Qtl[ot // 2][:, ot % 2, cs : cs + 512], st[:], qh_sl
                        )
                    if ci == 1:
                        nc.sync.dma_start(txxh[:], xxh[:])
                        nc.sync.dma_start(txxl[:], xxl[:])
                sc.__exit__(None, None, None)

            # =============================================================
            # Phases B/D (attention) + E
            # =============================================================
            attn_cm = tc.tile_pool(name="attn", bufs=1)
            attn = attn_cm.__enter__()
            # A tiles (2 phases x 4 kv-pairs)
            Ah2 = [
                [attn.tile([P, 2, H], F8, tag=f"Ah{p}_{j}", name=f"Ah{p}_{j}") for j in range(NP)]
                for p in range(2)
            ]
            Al2 = [
                [attn.tile([P, 2, H], F8, tag=f"Al{p}_{j}", name=f"Al{p}_{j}") for j in range(NP)]
                for p in range(2)
            ]
            # AX splits: [c-part, c-pair, q]
            AXh = [attn.tile([P, 2, H], F8, tag=f"AXh{j}", name=f"AXh{j}") for j in range(NP)]
            AXl = [attn.tile([P, 2, H], F8, tag=f"AXl{j}", name=f"AXl{j}") for j in range(NP)]
            rs_sb = attn.tile([P, H], FP, tag="rs_sb")
            rs_rc = attn.tile([P, H], FP, tag="rs_rc")

            def scores_phase(ph, s_list=None):
                Ah, Al = Ah2[ph], Al2[ph]
                txh = txoh if ph == 0 else txxh
                txl = txol if ph == 0 else txxl
                base = 512 * ph
                s_list = list(range(NT)) if s_list is None else s_list
                if 0 in s_list:
                    # zero the union-gap regions of the odd pair members
                    for js, soff in ((1, 0), (3, 256)):
                        g0 = base + soff
                        nc.gpsimd.memset(Ah[js][:, 1, g0 : g0 + P], 0)
                        nc.gpsimd.memset(Al[js][:, 1, g0 : g0 + P], 0)
                # chunk-column-major: all (lo,512) groups first, then all
                # (512,1024) — ph0's early groups need only ci=0 Qt evacs
                work = []
                for s in s_list:
                    lo = base + SLO[s] * P
                    for ck, (cs, ce) in enumerate(_chunks(lo, H)):
                        work.append((ck, s, cs, ce))
                work.sort(key=lambda t: (t[0], t[1]))
                for _, s, cs, ce in work:
                        w = ce - cs
                        ps = pp.tile([P, 512], FP, tag="ps", name=f"pss{ph}_{s}_{cs}")
                        n = 0
                        for kt, qt in ((txh, Qth), (txh, Qtl), (txl, Qth)):
                            for j in range(NP):
                                nc.tensor.matmul(
                                    ps[:, :w],
                                    lhsT=kt[:, j, :, s * P : (s + 1) * P],
                                    rhs=qt[j][:, :, cs:ce],
                                    start=(n == 0),
                                    stop=(n == 11),
                                    perf_mode=DR,
                                )
                                n += 1
                        for off, mname in MASKNAMES[s]:
                            a = base + off * P
                            if cs <= a < ce:
                                nc.vector.tensor_add(
                                    ps[:, a - cs : a - cs + P],
                                    ps[:, a - cs : a - cs + P],
                                    MT[mname][:],
                                )
                        e32 = stg.tile([P, 512], FP, tag="st", name=f"e32_{ph}_{s}_{cs}")
                        st2 = stg.tile([P, 512], FP, tag="st", name=f"st2_{ph}_{s}_{cs}")
                        nc.scalar.activation(
                            e32[:, :w], ps[:, :w], AF.Exp,
                            bias=ebias[:], scale=2.0**-9,
                        )
                        ah_sl = Ah[s // 2][:, s % 2, cs:ce]
                        nc.gpsimd.tensor_copy(ah_sl, e32[:, :w])
                        nc.vector.tensor_sub(st2[:, :w], e32[:, :w], ah_sl)
                        nc.vector.tensor_scalar_mul(
                            Al[s // 2][:, s % 2, cs:ce], st2[:, :w], 2.0**5
                        )

            def rowsum_phase(ph, cis=(0, 1)):
                # rowsum (consistent denominator): ones*Ah + ones5*Al.
                # Transient per-phase psums, accumulated in SBUF: ph0 owns
                # q-cols [0,512) outright (kv half 1 cannot attend them).
                Ah, Al = Ah2[ph], Al2[ph]
                base = 512 * ph
                for c_i in cis:
                    cl, cu = c_i * 512, (c_i + 1) * 512
                    segs = []
                    for ones_t, at in ((ones_h, Ah), (ones_l, Al)):
                        for js in range(NP):
                            lo = max(cl, base + PLO[js] * P)
                            if lo < cu:
                                segs.append((ones_t, at, js, lo))
                    if not segs:
                        continue
                    rps = pp.tile([P, 512], FP, tag="ps", name=f"rps{ph}_{c_i}")
                    for gi, (ones_t, at, js, lo) in enumerate(segs):
                        nc.tensor.matmul(
                            rps[:, lo - cl :],
                            lhsT=ones_t[:],
                            rhs=at[js][:, :, lo:cu],
                            start=(gi == 0),
                            stop=(gi == len(segs) - 1),
                            perf_mode=DR,
                        )
                    if ph == 0:
                        nc.scalar.activation(
                            rs_sb[:, cl:cu], rps[:], AF.Identity,
                            bias=zbias[:], scale=2.0**-2,
                        )
                    else:
                        rtmp = stg.tile([P, 512], FP, tag="st", name=f"rtmp{c_i}")
                        nc.scalar.activation(
                            rtmp[:], rps[:], AF.Identity,
                            bias=zbias[:], scale=2.0**-2,
                        )
                        nc.vector.tensor_add(
                            rs_sb[:, cl:cu], rs_sb[:, cl:cu], rtmp[:]
                        )

            def ax_split(ot, cs, ce, ps, norm=False):
                """psum (AX*2^2) -> AXh + boosted residual AXl.

                norm=False: AXh stores AX*2^-1 (scale-act evac).
                norm=True:  AXh stores (AX/den)*2^4 via rs_rc multiply —
                the z evac for these columns then needs no per-q divide."""
                st = ostg.tile([P, 512], FP, tag="ost", name=f"axst{ot}_{cs}")
                st2 = ostg.tile([P, 512], FP, tag="ost", name=f"axs2{ot}_{cs}")
                w = ce - cs
                axh_sl = AXh[ot // 2][:, ot % 2, cs:ce]
                if norm:
                    nc.vector.tensor_mul(st[:, :w], ps[:, :w], rs_rc[:, cs:ce])
                else:
                    nc.scalar.activation(
                        st[:, :w], ps[:, :w], AF.Identity,
                        bias=zbias[:], scale=2.0**-3,
                    )
                nc.gpsimd.tensor_copy(axh_sl, st[:, :w])
                nc.vector.tensor_sub(st2[:, :w], st[:, :w], axh_sl)
                nc.vector.tensor_scalar_mul(
                    AXl[ot // 2][:, ot % 2, cs:ce], st2[:, :w], 2.0**4
                )

            def ax_group_b2(ot):
                """AX q-cols [0,512): kv half 0 only."""
                osl = slice(ot * P, (ot + 1) * P)
                ps = pp.tile([P, 512], FP, tag="ps", name=f"psax0_{ot}")
                n_last = None
                groups = []
                for pi, (xt_t, at) in enumerate(
                    ((txtoh, Ah2[0]), (txtol, Ah2[0]), (txto5, Al2[0]))
                ):
                    for js in range(NP):
                        lo = PLO[js] * P
                        if lo >= 512:
                            continue
                        groups.append((xt_t, at, js, lo, pi))
                for gi, (xt_t, at, js, lo, pi) in enumerate(groups):
                    nc.tensor.matmul(
                        ps[:, lo:512],
                        lhsT=xt_t[:, js, :, osl],
                        rhs=at[js][:, :, lo:512],
                        start=(gi == 0),
                        stop=(gi == len(groups) - 1),
                        perf_mode=DR,
                    )
                ax_split(ot, 0, 512, ps)

            def ax_group_d2(ot, split=True):
                """AX q-cols [512,1024): full contraction over both kv halves.
                Splits rs-normalized (norm=True) so E1 needs no divide."""
                osl = slice(ot * P, (ot + 1) * P)
                ps = pp.tile([P, 512], FP, tag="ps", name=f"psax1_{ot}")
                groups = []
                for pi, (h_t, l_t, f_t) in enumerate(
                    ((txtoh, txtol, txto5), (txtxh, txtxl, txtx5))
                ):
                    at_h, at_l = Ah2[pi], Al2[pi]
                    base = 512 * pi
                    for xt_t, at in ((h_t, at_h), (l_t, at_h), (f_t, at_l)):
                        for js in range(NP):
                            lo = max(512, base + PLO[js] * P)
                            groups.append((xt_t, at, js, lo))
                for gi, (xt_t, at, js, lo) in enumerate(groups):
                    nc.tensor.matmul(
                        ps[:, lo - 512 :],
                        lhsT=xt_t[:, js, :, osl],
                        rhs=at[js][:, :, lo:H],
                        start=(gi == 0),
                        stop=(gi == len(groups) - 1),
                        perf_mode=DR,
                    )
                if split:
                    ax_split(ot, 512, H, ps)
                return ps

            def zproj_group(ci, o2, w=512, evac_dve=False):
                osl = slice(o2 * P, (o2 + 1) * P)
                for cs in range(ci * 512, (ci + 1) * 512, w):
                    ps = pp.tile([P, 512], FP, tag="ps", name=f"psz{o2}_{cs}")
                    for pi, (wt, ot_t) in enumerate(
                        ((tgh, AXh), (tgl, AXh), (tg5, AXl))
                    ):
                        for j in range(NP):
                            nc.tensor.matmul(
                                ps[:, :w],
                                lhsT=wt[:, j, :, osl],
                                rhs=ot_t[j][:, :, cs : cs + w],
                                start=(pi == 0 and j == 0),
                                stop=(pi == 2 and j == NP - 1),
                                perf_mode=DR,
                            )
                    st = ostg.tile([P, 512], FP, tag="ost", name=f"zst{o2}_{cs}")
                    nc.vector.tensor_mul(
                        st[:, :w], ps[:, :w], rs_rc[:, cs : cs + w]
                    )
                    ev = evac.tile([P, 512], BF, tag="evy")
                    if evac_dve:
                        nc.vector.tensor_scalar(
                            ev[:, :w], st[:, :w], 2.0**-8,
                            beff_sb[:, o2 : o2 + 1],
                            mybir.AluOpType.mult, mybir.AluOpType.add,
                        )
                    else:
                        nc.scalar.activation(
                            ev[:, :w], st[:, :w], AF.Identity,
                            bias=beff_sb[:, o2 : o2 + 1], scale=2.0**-8,
                        )
                    nc.sync.dma_start(
                        yT[
                            (o2 * 2 + ci) * P : (o2 * 2 + ci + 1) * P,
                            cs - ci * 512 : cs - ci * 512 + w,
                        ],
                        ev[:, :w],
                    )

            sc = tc.nc.named_scope("B1"); sc.__enter__()
            # kv-layout x loads overlap with B1 scores
            nc.sync.dma_start(txtoh[:], xtoh[:])
            nc.sync.dma_start(txtol[:], xtol[:])
            nc.sync.dma_start(txto5[:], xto5[:])
            scores_phase(0)
            nc.sync.dma_start(txtxh[:], xtxh[:])
            nc.sync.dma_start(txtxl[:], xtxl[:])
            nc.sync.dma_start(txtx5[:], xtx5[:])
            sc.__exit__(None, None, None)
            sc = tc.nc.named_scope("D1"); sc.__enter__()
            # rowsum ph0 emitted piecewise between D1 groups so PE is not
            # gated on B1's trailing evac chains (cols 512+ evac last)
            scores_phase(1, [0, 1])
            rowsum_phase(0, (0,))
            scores_phase(1, [2, 3, 4, 5, 6])
            rowsum_phase(0, (1,))
            scores_phase(1, [7])
            sc.__exit__(None, None, None)

            # G packs: DMA during B2 (needed by E0)
            tgh = big.tile([P, NP, 2, C], F8, tag="txoh", name="tgh")
            tgl = big.tile([P, NP, 2, C], F8, tag="txol", name="tgl")
            tg5 = big.tile([P, NP, 2, C], F8, tag="txxh", name="tg5")
            nc.sync.dma_start(tgh[:], gh[:])
            nc.sync.dma_start(tgl[:], gl[:])
            nc.sync.dma_start(tg5[:], g5[:])
            nc.sync.dma_start(beff_sb[:], beff[:])

            sc = tc.nc.named_scope("B2"); sc.__enter__()
            for ot in range(NT):
                ax_group_b2(ot)
            sc.__exit__(None, None, None)
            # E0 (zproj cols 0-511, inputs ready after B2) interleaves with
            # D2, offset by one so D2's last split chain is covered
            sc = tc.nc.named_scope("E0D2"); sc.__enter__()
            ax_group_d2(0)
            rowsum_phase(1)
            nc.vector.reciprocal(rs_rc[:], rs_sb[:])
            ax_group_d2(1)
            ax_group_d2(2)
            for i in range(3, NT):
                ax_group_d2(i)
                zproj_group(0, i - 3)
            for i in range(NT - 3, NT):
                zproj_group(0, i)
            sc.__exit__(None, None, None)
            sc = tc.nc.named_scope("E1"); sc.__enter__()
            for o2 in range(NT):
                zproj_group(1, o2, evac_dve=(o2 in (NT - 3, NT - 2)))
            sc.__exit__(None, None, None)
            attn_cm.__exit__(None, None, None)

    _split_waits(nc)
    return nc


_NC_CACHE = None


def _get_nc():
    global _NC_CACHE
    if _NC_CACHE is None:
        _NC_CACHE = _build_nc()
    return _NC_CACHE


def _split8(m, scale):
    """hi/lo fp8 split of m*scale (numpy, returns E4 arrays)."""
    s = np.asarray(m, np.float32) * np.float32(scale)
    hi = s.astype(E4)
    lo = (s - hi.astype(np.float32)).astype(E4)
    return hi, lo


def _pack4(m):
    """[1024, N] -> [128, 4*2*N]: partition p, cols (j, i, t) with
    source row 128*(2j+i)+p. Matches SBUF tile [P, NP, 2, N]."""
    n = m.shape[1]
    m4 = m.reshape(NP, 2, P, n).transpose(2, 0, 1, 3)
    return np.ascontiguousarray(m4).reshape(P, NP * 2 * n)


def make_in_maps(x, w_qkv, b_qkv, w_proj, b_proj):
    x = np.asarray(x, dtype=np.float32)
    w_qkv = np.asarray(w_qkv, dtype=np.float32)
    b_qkv = np.asarray(b_qkv, dtype=np.float32)
    w_proj = np.asarray(w_proj, dtype=np.float32)
    b_proj = np.asarray(b_proj, dtype=np.float32)

    s = np.float32(1.0 / np.sqrt(np.float32(C)))
    wq, wk, wv = w_qkv[0:C], w_qkv[C : 2 * C], w_qkv[2 * C :]
    bq, bk, bv = b_qkv[0:C], b_qkv[C : 2 * C], b_qkv[2 * C :]

    M = (s * (wq.T @ wk)).astype(np.float32)  # [c_q, c_kv]
    u = (s * (bq @ wk)).astype(np.float32)  # [c_kv]
    G = (w_proj @ wv).astype(np.float32)  # [o, c_kv]
    beff = np.ascontiguousarray((b_proj + w_proj @ bv).reshape(NT, P).T)

    mhp, mlp = (_pack4(a) for a in _split8(M, 2.0**12))
    ghp, glp = (_pack4(a) for a in _split8(G.T, 2.0**7))
    g5p = (ghp.astype(np.float32) * 2.0**-4).astype(E4)
    ub = np.ascontiguousarray((u * 2.0**7).reshape(NT, P).T)

    ones_h = np.ones((P, 2 * P), dtype=np.float32).astype(E4)
    ones_l = np.full((P, 2 * P), 2.0**-5, dtype=np.float32).astype(E4)

    triu = np.triu(np.ones((P, P), dtype=np.float32))
    trilm = np.where(triu > 0, 0.0, NEG).astype(np.float32)
    zeros = np.zeros((P, P), dtype=np.float32)
    negs = np.full((P, P), NEG, dtype=np.float32)

    shared = dict(
        mh=mhp.reshape(P, NP, 2, C), ml=mlp, gh=ghp, gl=glp, g5=g5p,
        ub=ub, beff=beff, ones_h=ones_h, ones_l=ones_l,
    )
    in_maps = []
    for core in range(8):
        b, h = core // 2, core % 2
        xb = x[b]  # [T, C]
        qrows = np.concatenate(
            [xb[(2 * bg + h) * 256 : (2 * bg + h + 1) * 256] for bg in range(4)],
            axis=0,
        )

        def cpack(rows):
            hi, lo = _split8(np.ascontiguousarray(rows.T), 2.0**2)
            return _pack4(hi), _pack4(lo)

        def kvpack(rows):
            hi, lo = _split8(np.ascontiguousarray(rows), 2.0**2)
            return _pack4(hi), _pack4(lo)

        xqh, xql = cpack(qrows)
        xoh, xol = cpack(xb[0:H])
        xxh, xxl = cpack(xb[H : 2 * H])
        xtoh, xtol = kvpack(xb[0:H])
        xtxh, xtxl = kvpack(xb[H : 2 * H])
        xto5 = (xtoh.astype(np.float32) * 2.0**-5).astype(E4)
        xtx5 = (xtxh.astype(np.float32) * 2.0**-5).astype(E4)
        in_maps.append(
            dict(
                shared,
                xqh=xqh.reshape(P, NP, 2, H), xql=xql,
                xoh=xoh, xol=xol, xxh=xxh, xxl=xxl,
                xtoh=xtoh, xtol=xtol, xtxh=xtxh, xtxl=xtxl,
                xto5=xto5, xtx5=xtx5,
                m1d_in=trilm if h == 0 else zeros,
                m1f_in=negs if h == 0 else zeros,
                m2d_in=negs if h == 0 else trilm,
            )
        )
    return in_maps


def assemble_output(results):
    B = 4
    y = np.empty((B, 2 * H, C), dtype=np.float32)
    for core in range(8):
        b, h = core // 2, core % 2
        yt = np.asarray(results[core]["yT"], dtype=np.float32).reshape(NT, 2, P, 512)
        blk = yt.transpose(1, 3, 0, 2).reshape(H, C)
        blk4 = blk.reshape(4, 256, C)
        for bg in range(4):
            g = 2 * bg + h
            y[b, g * 256 : (g + 1) * 256, :] = blk4[bg]
    return y


def kernel(x, w_qkv, b_qkv, w_proj, b_proj):
    from concourse.bass_utils import run_bass_kernel_spmd

    nc = _get_nc()
    in_maps = make_in_maps(x, w_qkv, b_qkv, w_proj, b_proj)
    res = run_bass_kernel_spmd(nc, in_maps, list(range(8)))
    return assemble_output(res.results)


# revision 61
# speedup vs baseline: 1.0052x; 1.0052x over previous
"""Single-head causal attention (B=4, T=2048, C=1024) on 8 trn2 NeuronCores.

Associativity-folded fp8 (e4m3) DoubleRow rewrite. Exploits two host-side
weight foldings so the C x C projections act on the SHARDED query side
(1024 rows/core) instead of the DUPLICATED kv side (2048 rows/core):

  softmax row-shift invariance  ->  bk drops out entirely
  S = (x_q M + u) x^T           with M = s Wq^T Wk,  u = s Wk^T bq
  y = (A x) G^T / den + beff    with G = Wp Wv,      beff = b_proj + Wp bv

K, V, Q, O never materialize. Per-core tensor work becomes
  Qt-proj (49k) + scores (58k) + rowsum (5k) + AX (61k) + z-proj (49k)
= 223k cost-model cycles (~93us) vs the baseline's 420k (~175us).

All matmul groups stay 3-term hi/lo fp8 DoubleRow (a*b ~= ah*bh + ah*bl +
al*bh) — the cheapest >=7-bit-effective scheme under the cost model.

Sharding: identical to baseline: 8 shards = (batch b) x (query interleave h),
query rows as interleaved 256-row blocks; S^T formulation (scores [kv, query])
so softmax denominators come from ones-matmuls and AX needs no transposes.

Scales (power-of-2):
  x*2^2 hi/lo;  M*2^12 hi/lo;  Qt stored *2^7 (u*2^7 bias);  S psum = S*2^9
  A = exp(S + EXPB) hi + boosted residual Al*2^5 (paired with xt5 = xth*2^-5
  and a 2^-5 ones tile);  AX psum = AX*2^2, stored AX*2^-1 hi + (res*2^4)
  paired with G5 = G*2^3;  G*2^7 hi/lo;  z psum = z*2^6;
  y = psum * (2^2/den) * 2^-8 + beff.

AX accumulation is psum-direct (no SBUF staging): phase B2 computes q-cols
[0,512) (kv half 0 is the only contributor there — causal interleave), phase
D2 computes q-cols [512,1024) with full contraction over both kv halves.
"""

import sys

sys.path.insert(0, "/opt/trn_rl_repo")

import numpy as np
import ml_dtypes

import concourse.bass as bass
import concourse.tile as tile
from concourse import mybir
from concourse.vector_clock import ScopedClock

FP = mybir.dt.float32
BF = mybir.dt.bfloat16
F8 = mybir.dt.float8e4
E4 = ml_dtypes.float8_e4m3
AF = mybir.ActivationFunctionType

P = 128
C = 1024  # embed dim
H = 1024  # query rows per core
NT = C // P  # 8 channel tiles
NP = 4  # channel pair-tiles
NEG = -1.0e12
EXPB = -3.11  # exp bias: A = exp(score + EXPB); max score 7.42 -> A_max ~74

_MAX_WAITS = 1


class _TC(tile.TileContext):
    """TileContext whose tail drain puts its global-clock waits on a nop
    (walrus rejects multi-wait Drain); excess waits are split by
    _split_waits() afterwards."""

    def _drain_and_barrier(self, tick_clock, wait_clock):
        nop_inst = self.nc.sync.nop(nofuse=True, hint="pre_drain_waits")
        wait_clock.add_sem_waits(
            nop_inst.ins, ScopedClock({None: tick_clock.global_clock})
        )
        self.nc.sync.drain()
        self.nc.all_engine_barrier()
        assert self.sems is not None
        popped = self.nc._tile_sem_poison_stack.pop()
        assert popped is self._sem_poison
        self.nc.clear_and_free_semaphores(list(self.sems.allocated().values()))


def _split_waits(nc, max_waits=_MAX_WAITS):
    """Move excess sync waits onto injected nops placed immediately before
    the instruction on the same engine (walrus rejects >1 wait/instruction)."""
    import copy

    template = nc.sync.nop(nofuse=True, hint="waitsplit_template").ins
    counter = [0]

    def make_nop(engine, waits):
        nop = copy.deepcopy(template)
        counter[0] += 1
        nop.name = f"I-wsplit-{counter[0]}"
        nop.engine = engine
        nop.sync_info = mybir.SyncInfo(on_wait=list(waits), on_update=[])
        return nop

    f = nc.m.functions[0]
    for bb in f.blocks:
        insts = bb.instructions
        if not any(
            i.sync_info and i.sync_info.on_wait and len(i.sync_info.on_wait) > max_waits
            for i in insts
        ):
            continue
        newlist = []
        for inst in insts:
            si = inst.sync_info
            if si and si.on_wait and len(si.on_wait) > max_waits:
                if inst.name == template.name:
                    newlist.append(inst)
                    continue
                waits = list(si.on_wait)
                del si.on_wait[max_waits:]
                rest = waits[max_waits:]
                while rest:
                    newlist.append(make_nop(inst.engine, rest[:max_waits]))
                    rest = rest[max_waits:]
            newlist.append(inst)
        bb.instructions[:] = newlist


def _chunks(lo, hi, step=512):
    out = []
    while lo < hi:
        w = min(step, hi - lo)
        out.append((lo, lo + w))
        lo += w
    return out


# Interleaved-256 balanced causal structure (see baseline docstring).
SLO = [0, 0, 0, 1, 2, 2, 2, 3]  # per kv 128-tile: first valid 128-col block
MASKNAMES = [
    [(0, "m1d")],
    [(0, "m1f"), (1, "m1d")],
    [(0, "m2d"), (1, "m1f")],
    [(1, "m2d")],
    [(2, "m1d")],
    [(2, "m1f"), (3, "m1d")],
    [(2, "m2d"), (3, "m1f")],
    [(3, "m2d")],
]
PLO = [0, 0, 2, 2]  # per kv-pair js: min(SLO[2js], SLO[2js+1])
DR = mybir.MatmulPerfMode.DoubleRow


def _build_nc():
    nc = bass.Bass("TRN2", target_bir_lowering=False, debug=False)

    def din(name, shape, dt=F8):
        return nc.dram_tensor(name, shape, dt, kind="ExternalInput").ap()

    # x packs, c-layout (contraction = channel): [128, NP*2*1024] j-major
    xqh, xql = din("xqh", [P, NP, 2, H]), din("xql", [P, NP * 2 * H])
    xoh, xol = din("xoh", [P, NP * 2 * H]), din("xol", [P, NP * 2 * H])
    xxh, xxl = din("xxh", [P, NP * 2 * H]), din("xxl", [P, NP * 2 * H])
    # x packs, kv-layout (contraction = kv token): for AX lhsT
    xtoh, xtol = din("xtoh", [P, NP * 2 * C]), din("xtol", [P, NP * 2 * C])
    xtxh, xtxl = din("xtxh", [P, NP * 2 * C]), din("xtxl", [P, NP * 2 * C])
    xto5, xtx5 = din("xto5", [P, NP * 2 * C]), din("xtx5", [P, NP * 2 * C])
    # folded weights
    mh, ml = din("mh", [P, NP, 2, C]), din("ml", [P, NP * 2 * C])
    gh, gl = din("gh", [P, NP * 2 * C]), din("gl", [P, NP * 2 * C])
    g5 = din("g5", [P, NP * 2 * C])
    ub = din("ub", [P, NT], FP)
    beff = din("beff", [P, NT], FP)
    ones_h_in = din("ones_h", [P, 2 * P])
    ones_l_in = din("ones_l", [P, 2 * P])
    m1d_in = din("m1d_in", [P, P], FP)
    m1f_in = din("m1f_in", [P, P], FP)
    m2d_in = din("m2d_in", [P, P], FP)
    # output in (o2-tile, chunk)-major layout; host reassembles
    yT = nc.dram_tensor("yT", [NT * 2 * P, 512], BF, kind="ExternalOutput").ap()

    with _TC(nc) as tc:
        with (
            tc.tile_pool(name="misc", bufs=1) as misc,
            tc.tile_pool(name="big", bufs=1) as big,
            tc.tile_pool(name="stg", bufs=5) as stg,
            tc.tile_pool(name="ostg", bufs=4) as ostg,
            tc.tile_pool(name="evac", bufs=4) as evac,
            tc.tile_pool(name="psum", bufs=8, space="PSUM") as pp,
        ):
            ones_h = misc.tile([P, 2, P], F8, tag="ones_h")
            ones_l = misc.tile([P, 2, P], F8, tag="ones_l")
            m1d = misc.tile([P, P], FP, tag="m1d")
            m1f = misc.tile([P, P], FP, tag="m1f")
            m2d = misc.tile([P, P], FP, tag="m2d")
            ub_sb = misc.tile([P, NT], FP, tag="ub")
            beff_sb = misc.tile([P, NT], FP, tag="beff")
            ebias = misc.tile([P, 1], FP, tag="ebias")
            zbias = misc.tile([P, 1], FP, tag="zbias")
            nc.gpsimd.memset(ebias[:], EXPB)
            nc.gpsimd.memset(zbias[:], 0.0)

            MT = {"m1d": m1d, "m1f": m1f, "m2d": m2d}

            # ---- persistent tensors ------------------------------------
            # Qt: [c-part, c-pair, q]  (rhs of scores)
            Qth = [big.tile([P, 2, H], F8, tag=f"Qth{j}", name=f"Qth{j}") for j in range(NP)]
            Qtl = [big.tile([P, 2, H], F8, tag=f"Qtl{j}", name=f"Qtl{j}") for j in range(NP)]
            # x c-layout (scores lhsT), 4 j-tiles per pack held as one tile
            txoh = big.tile([P, NP, 2, H], F8, tag="txoh", name="txoh")
            txol = big.tile([P, NP, 2, H], F8, tag="txol", name="txol")
            txxh = big.tile([P, NP, 2, H], F8, tag="txxh", name="txxh")
            txxl = big.tile([P, NP, 2, H], F8, tag="txxl", name="txxl")
            # x kv-layout (AX lhsT)
            txtoh = big.tile([P, NP, 2, C], F8, tag="txtoh", name="txtoh")
            txtol = big.tile([P, NP, 2, C], F8, tag="txtol", name="txtol")
            txtxh = big.tile([P, NP, 2, C], F8, tag="txtxh", name="txtxh")
            txtxl = big.tile([P, NP, 2, C], F8, tag="txtxl", name="txtxl")
            txto5 = big.tile([P, NP, 2, C], F8, tag="txto5", name="txto5")
            txtx5 = big.tile([P, NP, 2, C], F8, tag="txtx5", name="txtx5")
            # =============================================================
            # Phase A: Qt projection  (Qt = x_q M + u)
            # =============================================================
            with tc.tile_pool(name="aw", bufs=1) as aw:
                tmh = [aw.tile([P, 2, C], F8, tag=f"tmh{j}", name=f"tmh{j}") for j in range(NP)]
                tml = [aw.tile([P, 2, C], F8, tag=f"tml{j}", name=f"tml{j}") for j in range(NP)]
                txqh = [aw.tile([P, 2, H], F8, tag=f"txqh{j}", name=f"txqh{j}") for j in range(NP)]
                txql = [aw.tile([P, 2, H], F8, tag=f"txql{j}", name=f"txql{j}") for j in range(NP)]

                # DMA order matches matmul consumption: first operands as
                # slivers for an early start; tml rides each j-wave so every
                # arriving tile unlocks work in all 8 open psum groups
                # first two transfers on separate issue queues (SP + Act)
                # so their HWDGE setups overlap
                nc.sync.dma_start(tmh[0][:, :, 0:128], mh[:, 0, :, 0:128])
                nc.scalar.dma_start(txqh[0][:, :, 0:512], xqh[:, 0, :, 0:512])
                nc.sync.dma_start(tmh[0][:, :, 128:C], mh[:, 0, :, 128:C])
                nc.sync.dma_start(txqh[0][:, :, 512:H], xqh[:, 0, :, 512:H])
                nc.sync.dma_start(tml[0][:], ml[:, 0 : 2 * C])
                for j in range(1, NP):
                    nc.sync.dma_start(tmh[j][:], mh[:, j, :, :])
                    nc.sync.dma_start(txqh[j][:], xqh[:, j, :, :])
                    nc.sync.dma_start(tml[j][:], ml[:, j * 2 * C : (j + 1) * 2 * C])
                nc.sync.dma_start(ub_sb[:], ub[:])
                for j in range(NP):
                    nc.sync.dma_start(txql[j][:], xql[:, j * 2 * H : (j + 1) * 2 * H])
                # B1 operands trickle in behind the A-phase packs
                nc.sync.dma_start(ones_h[:], ones_h_in[:])
                nc.sync.dma_start(ones_l[:], ones_l_in[:])
                nc.sync.dma_start(m1d[:], m1d_in[:])
                nc.sync.dma_start(m1f[:], m1f_in[:])
                nc.sync.dma_start(m2d[:], m2d_in[:])
                nc.sync.dma_start(txoh[:], xoh[:])
                nc.sync.dma_start(txol[:], xol[:])

                sc = tc.nc.named_scope("A_qt"); sc.__enter__()
                for ci in range(2):
                    cs = ci * 512
                    for ot in range(NT):
                        colsl = slice(ot * P, (ot + 1) * P)
                        ps = pp.tile([P, 512], FP, tag="ps", name=f"psqt{ci}_{ot}")
                        # j-major over the M hi/lo terms to match the DMA
                        # wave order (tmh_j, txqh_j, tml_j), txql terms last
                        terms = [(tmh, txqh, j) for j in range(NP)]
                        terms = [t for j in range(NP)
                                 for t in ((tmh, txqh, j), (tml, txqh, j))]
                        terms += [(tmh, txql, j) for j in range(NP)]
                        for n, (wt, xt, j) in enumerate(terms):
                            nc.tensor.matmul(
                                ps[:],
                                lhsT=wt[j][:, :, colsl],
                                rhs=xt[j][:, :, cs : cs + 512],
                                start=(n == 0),
                                stop=(n == 11),
                                perf_mode=DR,
                            )
                        st = stg.tile([P, 512], FP, tag="st", name=f"stqt{ci}_{ot}")
                        qh_sl = Qth[ot // 2][:, ot % 2, cs : cs + 512]
                        nc.scalar.activation(
                            st[:], ps[:], AF.Identity,
                            bias=ub_sb[:, ot : ot + 1], scale=2.0**-7,
                        )
                        nc.gpsimd.tensor_copy(qh_sl, st[:])
                        nc.vector.tensor_sub(
                            Qtl[ot // 2][:, ot % 2, cs : cs + 512], st[:], qh_sl
                        )
                    if ci == 1:
                        nc.sync.dma_start(txxh[:], xxh[:])
                        nc.sync.dma_start(txxl[:], xxl[:])
                sc.__exit__(None, None, None)

            # =============================================================
            # Phases B/D (attention) + E
            # =============================================================
            attn_cm = tc.tile_pool(name="attn", bufs=1)
            attn = attn_cm.__enter__()
            # A tiles (2 phases x 4 kv-pairs)
            Ah2 = [
                [attn.tile([P, 2, H], F8, tag=f"Ah{p}_{j}", name=f"Ah{p}_{j}") for j in range(NP)]
                for p in range(2)
            ]
            Al2 = [
                [attn.tile([P, 2, H], F8, tag=f"Al{p}_{j}", name=f"Al{p}_{j}") for j in range(NP)]
                for p in range(2)
            ]
            # AX splits: [c-part, c-pair, q]
            AXh = [attn.tile([P, 2, H], F8, tag=f"AXh{j}", name=f"AXh{j}") for j in range(NP)]
            AXl = [attn.tile([P, 2, H], F8, tag=f"AXl{j}", name=f"AXl{j}") for j in range(NP)]
            rs_sb = attn.tile([P, H], FP, tag="rs_sb")
            rs_rc = attn.tile([P, H], FP, tag="rs_rc")

            def scores_phase(ph, s_list=None):
                Ah, Al = Ah2[ph], Al2[ph]
                txh = txoh if ph == 0 else txxh
                txl = txol if ph == 0 else txxl
                base = 512 * ph
                s_list = list(range(NT)) if s_list is None else s_list
                if 0 in s_list:
                    # zero the union-gap regions of the odd pair members
                    for js, soff in ((1, 0), (3, 256)):
                        g0 = base + soff
                        nc.gpsimd.memset(Ah[js][:, 1, g0 : g0 + P], 0)
                        nc.gpsimd.memset(Al[js][:, 1, g0 : g0 + P], 0)
                # chunk-column-major: all (lo,512) groups first, then all
                # (512,1024) — ph0's early groups need only ci=0 Qt evacs
                work = []
                for s in s_list:
                    lo = base + SLO[s] * P
                    for ck, (cs, ce) in enumerate(_chunks(lo, H)):
                        work.append((ck, s, cs, ce))
                work.sort(key=lambda t: (t[0], t[1]))
                for _, s, cs, ce in work:
                        w = ce - cs
                        ps = pp.tile([P, 512], FP, tag="ps", name=f"pss{ph}_{s}_{cs}")
                        n = 0
                        for kt, qt in ((txh, Qth), (txh, Qtl), (txl, Qth)):
                            for j in range(NP):
                                nc.tensor.matmul(
                                    ps[:, :w],
                                    lhsT=kt[:, j, :, s * P : (s + 1) * P],
                                    rhs=qt[j][:, :, cs:ce],
                                    start=(n == 0),
                                    stop=(n == 11),
                                    perf_mode=DR,
                                )
                                n += 1
                        for off, mname in MASKNAMES[s]:
                            a = base + off * P
                            if cs <= a < ce:
                                nc.vector.tensor_add(
                                    ps[:, a - cs : a - cs + P],
                                    ps[:, a - cs : a - cs + P],
                                    MT[mname][:],
                                )
                        e32 = stg.tile([P, 512], FP, tag="st", name=f"e32_{ph}_{s}_{cs}")
                        st2 = stg.tile([P, 512], FP, tag="st", name=f"st2_{ph}_{s}_{cs}")
                        nc.scalar.activation(
                            e32[:, :w], ps[:, :w], AF.Exp,
                            bias=ebias[:], scale=2.0**-9,
                        )
                        ah_sl = Ah[s // 2][:, s % 2, cs:ce]
                        nc.gpsimd.tensor_copy(ah_sl, e32[:, :w])
                        nc.vector.tensor_sub(st2[:, :w], e32[:, :w], ah_sl)
                        nc.vector.tensor_scalar_mul(
                            Al[s // 2][:, s % 2, cs:ce], st2[:, :w], 2.0**5
                        )

            def rowsum_phase(ph, cis=(0, 1)):
                # rowsum (consistent denominator): ones*Ah + ones5*Al.
                # Transient per-phase psums, accumulated in SBUF: ph0 owns
                # q-cols [0,512) outright (kv half 1 cannot attend them).
                Ah, Al = Ah2[ph], Al2[ph]
                base = 512 * ph
                for c_i in cis:
                    cl, cu = c_i * 512, (c_i + 1) * 512
                    segs = []
                    for ones_t, at in ((ones_h, Ah), (ones_l, Al)):
                        for js in range(NP):
                            lo = max(cl, base + PLO[js] * P)
                            if lo < cu:
                                segs.append((ones_t, at, js, lo))
                    if not segs:
                        continue
                    rps = pp.tile([P, 512], FP, tag="ps", name=f"rps{ph}_{c_i}")
                    for gi, (ones_t, at, js, lo) in enumerate(segs):
                        nc.tensor.matmul(
                            rps[:, lo - cl :],
                            lhsT=ones_t[:],
                            rhs=at[js][:, :, lo:cu],
                            start=(gi == 0),
                            stop=(gi == len(segs) - 1),
                            perf_mode=DR,
                        )
                    if ph == 0:
                        nc.scalar.activation(
                            rs_sb[:, cl:cu], rps[:], AF.Identity,
                            bias=zbias[:], scale=2.0**-2,
                        )
                    else:
                        rtmp = stg.tile([P, 512], FP, tag="st", name=f"rtmp{c_i}")
                        nc.scalar.activation(
                            rtmp[:], rps[:], AF.Identity,
                            bias=zbias[:], scale=2.0**-2,
                        )
                        nc.vector.tensor_add(
                            rs_sb[:, cl:cu], rs_sb[:, cl:cu], rtmp[:]
                        )

            def ax_split(ot, cs, ce, ps, norm=False):
                """psum (AX*2^2) -> AXh + boosted residual AXl.

                norm=False: AXh stores AX*2^-1 (scale-act evac).
                norm=True:  AXh stores (AX/den)*2^4 via rs_rc multiply —
                the z evac for these columns then needs no per-q divide."""
                st = ostg.tile([P, 512], FP, tag="ost", name=f"axst{ot}_{cs}")
                st2 = ostg.tile([P, 512], FP, tag="ost", name=f"axs2{ot}_{cs}")
                w = ce - cs
                axh_sl = AXh[ot // 2][:, ot % 2, cs:ce]
                if norm:
                    nc.vector.tensor_mul(st[:, :w], ps[:, :w], rs_rc[:, cs:ce])
                else:
                    nc.scalar.activation(
                        st[:, :w], ps[:, :w], AF.Identity,
                        bias=zbias[:], scale=2.0**-3,
                    )
                nc.gpsimd.tensor_copy(axh_sl, st[:, :w])
                nc.vector.tensor_sub(st2[:, :w], st[:, :w], axh_sl)
                nc.vector.tensor_scalar_mul(
                    AXl[ot // 2][:, ot % 2, cs:ce], st2[:, :w], 2.0**4
                )

            def ax_group_b2(ot):
                """AX q-cols [0,512): kv half 0 only."""
                osl = slice(ot * P, (ot + 1) * P)
                ps = pp.tile([P, 512], FP, tag="ps", name=f"psax0_{ot}")
                n_last = None
                groups = []
                for pi, (xt_t, at) in enumerate(
                    ((txtoh, Ah2[0]), (txtol, Ah2[0]), (txto5, Al2[0]))
                ):
                    for js in range(NP):
                        lo = PLO[js] * P
                        if lo >= 512:
                            continue
                        groups.append((xt_t, at, js, lo, pi))
                for gi, (xt_t, at, js, lo, pi) in enumerate(groups):
                    nc.tensor.matmul(
                        ps[:, lo:512],
                        lhsT=xt_t[:, js, :, osl],
                        rhs=at[js][:, :, lo:512],
                        start=(gi == 0),
                        stop=(gi == len(groups) - 1),
                        perf_mode=DR,
                    )
                ax_split(ot, 0, 512, ps)

            def ax_group_d2(ot, split=True):
                """AX q-cols [512,1024): full contraction over both kv halves.
                Splits rs-normalized (norm=True) so E1 needs no divide."""
                osl = slice(ot * P, (ot + 1) * P)
                ps = pp.tile([P, 512], FP, tag="ps", name=f"psax1_{ot}")
                groups = []
                for pi, (h_t, l_t, f_t) in enumerate(
                    ((txtoh, txtol, txto5), (txtxh, txtxl, txtx5))
                ):
                    at_h, at_l = Ah2[pi], Al2[pi]
                    base = 512 * pi
                    for xt_t, at in ((h_t, at_h), (l_t, at_h), (f_t, at_l)):
                        for js in range(NP):
                            lo = max(512, base + PLO[js] * P)
                            groups.append((xt_t, at, js, lo))
                for gi, (xt_t, at, js, lo) in enumerate(groups):
                    nc.tensor.matmul(
                        ps[:, lo - 512 :],
                        lhsT=xt_t[:, js, :, osl],
                        rhs=at[js][:, :, lo:H],
                        start=(gi == 0),
                        stop=(gi == len(groups) - 1),
                        perf_mode=DR,
                    )
                if split:
                    ax_split(ot, 512, H, ps)
                return ps

            def zproj_group(ci, o2, w=512, evac_dve=False):
                osl = slice(o2 * P, (o2 + 1) * P)
                for cs in range(ci * 512, (ci + 1) * 512, w):
                    ps = pp.tile([P, 512], FP, tag="ps", name=f"psz{o2}_{cs}")
                    for pi, (wt, ot_t) in enumerate(
                        ((tgh, AXh), (tgl, AXh), (tg5, AXl))
                    ):
                        for j in range(NP):
                            nc.tensor.matmul(
                                ps[:, :w],
                                lhsT=wt[:, j, :, osl],
                                rhs=ot_t[j][:, :, cs : cs + w],
                                start=(pi == 0 and j == 0),
                                stop=(pi == 2 and j == NP - 1),
                                perf_mode=DR,
                            )
                    st = ostg.tile([P, 512], FP, tag="ost", name=f"zst{o2}_{cs}")
                    nc.vector.tensor_mul(
                        st[:, :w], ps[:, :w], rs_rc[:, cs : cs + w]
                    )
                    ev = evac.tile([P, 512], BF, tag="evy")
                    if evac_dve:
                        nc.vector.tensor_scalar(
                            ev[:, :w], st[:, :w], 2.0**-8,
                            beff_sb[:, o2 : o2 + 1],
                            mybir.AluOpType.mult, mybir.AluOpType.add,
                        )
                    else:
                        nc.scalar.activation(
                            ev[:, :w], st[:, :w], AF.Identity,
                            bias=beff_sb[:, o2 : o2 + 1], scale=2.0**-8,
                        )
                    nc.sync.dma_start(
                        yT[
                            (o2 * 2 + ci) * P : (o2 * 2 + ci + 1) * P,
                            cs - ci * 512 : cs - ci * 512 + w,
                        ],
                        ev[:, :w],
                    )

            sc = tc.nc.named_scope("B1"); sc.__enter__()
            # kv-layout x loads overlap with B1 scores
            nc.sync.dma_start(txtoh[:], xtoh[:])
            nc.sync.dma_start(txtol[:], xtol[:])
            nc.sync.dma_start(txto5[:], xto5[:])
            scores_phase(0)
            nc.sync.dma_start(txtxh[:], xtxh[:])
            nc.sync.dma_start(txtxl[:], xtxl[:])
            nc.sync.dma_start(txtx5[:], xtx5[:])
            sc.__exit__(None, None, None)
            sc = tc.nc.named_scope("D1"); sc.__enter__()
            # rowsum ph0 emitted piecewise between D1 groups so PE is not
            # gated on B1's trailing evac chains (cols 512+ evac last)
            scores_phase(1, [0, 1])
            rowsum_phase(0, (0,))
            scores_phase(1, [2, 3, 4, 5, 6])
            rowsum_phase(0, (1,))
            scores_phase(1, [7])
            sc.__exit__(None, None, None)

            # G packs: DMA during B2 (needed by E0)
            tgh = big.tile([P, NP, 2, C], F8, tag="txoh", name="tgh")
            tgl = big.tile([P, NP, 2, C], F8, tag="txol", name="tgl")
            tg5 = big.tile([P, NP, 2, C], F8, tag="txxh", name="tg5")
            nc.sync.dma_start(tgh[:], gh[:])
            nc.sync.dma_start(tgl[:], gl[:])
            nc.sync.dma_start(tg5[:], g5[:])
            nc.sync.dma_start(beff_sb[:], beff[:])

            sc = tc.nc.named_scope("B2"); sc.__enter__()
            for ot in range(NT):
                ax_group_b2(ot)
            sc.__exit__(None, None, None)
            # E0 (zproj cols 0-511, inputs ready after B2) interleaves with
            # D2, offset by one so D2's last split chain is covered
            sc = tc.nc.named_scope("E0D2"); sc.__enter__()
            ax_group_d2(0)
            rowsum_phase(1)
            nc.vector.reciprocal(rs_rc[:], rs_sb[:])
            ax_group_d2(1)
            ax_group_d2(2)
            for i in range(3, NT):
                ax_group_d2(i)
                zproj_group(0, i - 3)
            for i in range(NT - 3, NT):
                zproj_group(0, i)
            sc.__exit__(None, None, None)
            sc = tc.nc.named_scope("E1"); sc.__enter__()
            for o2 in range(NT):
                zproj_group(1, o2, evac_dve=(o2 >= NT - 3))
            sc.__exit__(None, None, None)
            attn_cm.__exit__(None, None, None)

    _split_waits(nc)
    return nc


_NC_CACHE = None


def _get_nc():
    global _NC_CACHE
    if _NC_CACHE is None:
        _NC_CACHE = _build_nc()
    return _NC_CACHE


def _split8(m, scale):
    """hi/lo fp8 split of m*scale (numpy, returns E4 arrays)."""
    s = np.asarray(m, np.float32) * np.float32(scale)
    hi = s.astype(E4)
    lo = (s - hi.astype(np.float32)).astype(E4)
    return hi, lo


def _pack4(m):
    """[1024, N] -> [128, 4*2*N]: partition p, cols (j, i, t) with
    source row 128*(2j+i)+p. Matches SBUF tile [P, NP, 2, N]."""
    n = m.shape[1]
    m4 = m.reshape(NP, 2, P, n).transpose(2, 0, 1, 3)
    return np.ascontiguousarray(m4).reshape(P, NP * 2 * n)


def make_in_maps(x, w_qkv, b_qkv, w_proj, b_proj):
    x = np.asarray(x, dtype=np.float32)
    w_qkv = np.asarray(w_qkv, dtype=np.float32)
    b_qkv = np.asarray(b_qkv, dtype=np.float32)
    w_proj = np.asarray(w_proj, dtype=np.float32)
    b_proj = np.asarray(b_proj, dtype=np.float32)

    s = np.float32(1.0 / np.sqrt(np.float32(C)))
    wq, wk, wv = w_qkv[0:C], w_qkv[C : 2 * C], w_qkv[2 * C :]
    bq, bk, bv = b_qkv[0:C], b_qkv[C : 2 * C], b_qkv[2 * C :]

    M = (s * (wq.T @ wk)).astype(np.float32)  # [c_q, c_kv]
    u = (s * (bq @ wk)).astype(np.float32)  # [c_kv]
    G = (w_proj @ wv).astype(np.float32)  # [o, c_kv]
    beff = np.ascontiguousarray((b_proj + w_proj @ bv).reshape(NT, P).T)

    mhp, mlp = (_pack4(a) for a in _split8(M, 2.0**12))
    ghp, glp = (_pack4(a) for a in _split8(G.T, 2.0**7))
    g5p = (ghp.astype(np.float32) * 2.0**-4).astype(E4)
    ub = np.ascontiguousarray((u * 2.0**7).reshape(NT, P).T)

    ones_h = np.ones((P, 2 * P), dtype=np.float32).astype(E4)
    ones_l = np.full((P, 2 * P), 2.0**-5, dtype=np.float32).astype(E4)

    triu = np.triu(np.ones((P, P), dtype=np.float32))
    trilm = np.where(triu > 0, 0.0, NEG).astype(np.float32)
    zeros = np.zeros((P, P), dtype=np.float32)
    negs = np.full((P, P), NEG, dtype=np.float32)

    shared = dict(
        mh=mhp.reshape(P, NP, 2, C), ml=mlp, gh=ghp, gl=glp, g5=g5p,
        ub=ub, beff=beff, ones_h=ones_h, ones_l=ones_l,
    )
    in_maps = []
    for core in range(8):
        b, h = core // 2, core % 2
        xb = x[b]  # [T, C]
        qrows = np.concatenate(
            [xb[(2 * bg + h) * 256 : (2 * bg + h + 1) * 256] for bg in range(4)],
            axis=0,
        )

        def cpack(rows):
            hi, lo = _split8(np.ascontiguousarray(rows.T), 2.0**2)
            return _pack4(hi), _pack4(lo)

        def kvpack(rows):
            hi, lo = _split8(np.ascontiguousarray(rows), 2.0**2)
            return _pack4(hi), _pack4(lo)

        xqh, xql = cpack(qrows)
        xoh, xol = cpack(xb[0:H])
        xxh, xxl = cpack(xb[H : 2 * H])
        xtoh, xtol = kvpack(xb[0:H])
        xtxh, xtxl = kvpack(xb[H : 2 * H])
        xto5 = (xtoh.astype(np.float32) * 2.0**-5).astype(E4)
        xtx5 = (xtxh.astype(np.float32) * 2.0**-5).astype(E4)
        in_maps.append(
            dict(
                shared,
                xqh=xqh.reshape(P, NP, 2, H), xql=xql,
                xoh=xoh, xol=xol, xxh=xxh, xxl=xxl,
                xtoh=xtoh, xtol=xtol, xtxh=xtxh, xtxl=xtxl,
                xto5=xto5, xtx5=xtx5,
                m1d_in=trilm if h == 0 else zeros,
                m1f_in=negs if h == 0 else zeros,
                m2d_in=negs if h == 0 else trilm,
            )
        )
    return in_maps


def assemble_output(results):
    B = 4
    y = np.empty((B, 2 * H, C), dtype=np.float32)
    for core in range(8):
        b, h = core // 2, core % 2
        yt = np.asarray(results[core]["yT"], dtype=np.float32).reshape(NT, 2, P, 512)
        blk = yt.transpose(1, 3, 0, 2).reshape(H, C)
        blk4 = blk.reshape(4, 256, C)
        for bg in range(4):
            g = 2 * bg + h
            y[b, g * 256 : (g + 1) * 256, :] = blk4[bg]
    return y


def kernel(x, w_qkv, b_qkv, w_proj, b_proj):
    from concourse.bass_utils import run_bass_kernel_spmd

    nc = _get_nc()
    in_maps = make_in_maps(x, w_qkv, b_qkv, w_proj, b_proj)
    res = run_bass_kernel_spmd(nc, in_maps, list(range(8)))
    return assemble_output(res.results)
